# revision 21
# baseline (speedup 1.0000x reference)
"""BottleneckAttention3D kernel for 8 Trainium2 NeuronCores.

Reference computation (per batch b):
    h = GroupNorm(x)                      # [C, N], C=128, N=4096, 8 groups
    q = wq @ h + bq ; k = wk @ h + bk ; v = wv @ h + bv
    attn = softmax(q.T k / sqrt(C))       # [N, N]
    out = v attn.T ; y = x + wp @ out + bp

Sharding: 8 cores = 2 batches x 4 query blocks of NQ=1024 tokens. Each core
computes K/V for its whole batch and Q for its query block, then runs a
flash-attention-style loop over 32 key blocks of 128 tokens; the N^2 score
matrix lives only in PSUM/SBUF.

Host preprocessing (cheap, 0.2% of FLOPs): groupnorm statistics and the
affine fold into the QKV weights (W' = W diag(s), b' = W t + b), plus
weight transposes and fp16 casts of x.

Device-side structure per core:
  * K = Wk' x, V = (Wv' x)^T, q^T = Wq'' x_s + bq'' as fp16 tiles. The K
    bias is dropped entirely: softmax is invariant to per-query shifts.
    The V bias reduces to an additive constant (rows of attn sum to 1),
    folded into the projection bias on host.
  * Main loop (software-pipelined): scoresT block = K-block^T Q (fp16
    matmuls, f32 PSUM) -> exp on ACT (no max subtraction; scores are O(6))
    -> fp16 E tile -> attention*V accumulated in PSUM. The softmax
    denominator sum(E) accumulates on the Vector and GpSimd engines.
  * 1/d = exp(-ln d) on ACT (same table set as Exp), broadcast across
    partitions with a K=1 ones matmul, projection + residual, one DMA out.
"""

import sys

sys.path.insert(0, "/opt/trn_rl_repo")

import numpy as np

B = 2
C = 128
N = 4096  # 16*16*16 tokens
NQ = N // 4  # query block per core (1024)
GROUPS = 8
EPS = 1e-5
XCH = 1024
NX = N // XCH  # 4
MB = N // 128  # 32 key blocks
# denominator accumulation: blocks with (i % 8) < DVE_MOD go to the vector
# engine, the rest to gpsimd
DVE_MOD = 5

_CACHE = {}


def _build():
    import concourse.bacc as bacc
    import concourse.mybir as mybir
    import concourse.tile as tile

    F32 = mybir.dt.float32
    F32R = mybir.dt.float32r
    F16 = mybir.dt.float16
    Exp = mybir.ActivationFunctionType.Exp
    Ln = mybir.ActivationFunctionType.Ln

    nc = bacc.Bacc("TRN2", target_bir_lowering=False, debug=False)

    # ---- DRAM I/O ----
    xh_d = nc.dram_tensor("xh", [C, N], F16, kind="ExternalInput")
    xsh_d = nc.dram_tensor("xsh", [C, NQ], F16, kind="ExternalInput")
    xs_d = nc.dram_tensor("xs", [C, NQ], F32, kind="ExternalInput")
    wf_d = nc.dram_tensor("wf", [C, 3 * C], F16, kind="ExternalInput")  # wq|wk|wv
    wpt_d = nc.dram_tensor("wpt", [C, C], F32R, kind="ExternalInput")
    cols_d = nc.dram_tensor("cols", [C, 1], F32, kind="ExternalInput")  # fb
    bt_d = nc.dram_tensor("bt", [C, MB], F32, kind="ExternalInput")  # k.bq'' per block
    onc_d = nc.dram_tensor("onc", [C, 1], F32R, kind="ExternalInput")
    onh_d = nc.dram_tensor("onh", [C, 1], F16, kind="ExternalInput")
    onr_d = nc.dram_tensor("onr", [1, C], F32R, kind="ExternalInput")
    y_d = nc.dram_tensor("y", [C, NQ], F32, kind="ExternalOutput")

    with tile.TileContext(nc) as tc:
        with (
            tc.tile_pool(name="cst", bufs=1) as cst,
            tc.tile_pool(name="xp", bufs=1) as xp,
            tc.tile_pool(name="ep", bufs=10) as ep,
            tc.tile_pool(name="psm", bufs=2, space="PSUM") as psm,
            tc.tile_pool(name="pso", bufs=1, space="PSUM") as pso,
        ):
            # dummy ACT op: load the ln+exp table set at t=0
            DUM = cst.tile([1, 1], F32, tag="dum")
            nc.vector.memset(DUM, 1.0)
            DUM2 = cst.tile([1, 1], F32, tag="dum2")
            nc.scalar.activation(DUM2, DUM, Exp)

            # ---- input loads ----
            XH = []
            for j in range(NX):
                xt = xp.tile([C, XCH], F16, tag=f"x{j}", name=f"x{j}")
                nc.sync.dma_start(xt, xh_d[:, j * XCH : (j + 1) * XCH])
                XH.append(xt)
            XSH = cst.tile([C, NQ], F16, tag="xsh")
            nc.sync.dma_start(XSH, xsh_d[:, :])
            XS = cst.tile([C, NQ], F32, tag="xs")
            nc.sync.dma_start(XS, xs_d[:, :])
            WF = cst.tile([C, 3 * C], F16, tag="wf")
            nc.gpsimd.dma_start(WF, wf_d[:, :])
            WPT = cst.tile([C, C], F32R, tag="wpt")
            nc.gpsimd.dma_start(WPT, wpt_d[:, :])
            COLS = cst.tile([C, 1], F32, tag="cols")
            nc.gpsimd.dma_start(COLS, cols_d[:, :])
            BT = cst.tile([C, MB], F32, tag="bt")
            nc.gpsimd.dma_start(BT, bt_d[:, :])
            ONC = cst.tile([C, 1], F32R, tag="onc")
            nc.gpsimd.dma_start(ONC, onc_d[:, :])
            ONH = cst.tile([C, 1], F16, tag="onh")
            nc.gpsimd.dma_start(ONH, onh_d[:, :])
            ONR = cst.tile([1, C], F32R, tag="onr")
            nc.gpsimd.dma_start(ONR, onr_d[:, :])
            WQF = WF[:, 0 * C : 1 * C]
            WKF = WF[:, 1 * C : 2 * C]
            WVF = WF[:, 2 * C : 3 * C]
            FB = COLS[:, 0:1]

            XSB = cst.tile([C, NQ], F32, tag="xsb")
            nc.vector.tensor_scalar_add(XSB, XS, FB)

            # ---- Q then K (fp16; k-bias dropped: softmax shift-invariant) ----
            PQ = psm.tile([C, NQ], F32, tag="psq", name="pq")
            for h in range(2):
                sl = slice(h * 512, (h + 1) * 512)
                nc.tensor.matmul(PQ[:, sl], WQF, XSH[:, sl], start=True, stop=True)
            QT = cst.tile([C, NQ], F16, tag="qt")
            nc.vector.tensor_copy(QT, PQ)
            K = []
            for j2 in range(2 * NX):
                pk = psm.tile([C, 512], F32, tag="ps", name=f"pk{j2}")
                nc.tensor.matmul(
                    pk, WKF, XH[j2 // 2][:, (j2 % 2) * 512 : (j2 % 2 + 1) * 512],
                    start=True, stop=True,
                )
                kt = xp.tile([C, 512], F16, tag=f"k{j2}", name=f"k{j2}")
                nc.vector.tensor_copy(kt, pk)
                K.append(kt)
            V = [None] * (2 * NX)

            # ---- main attention loop ----
            PO = pso.tile([C, NQ], F32, tag="po")
            ACCD = [
                cst.tile([C, NQ], F32, tag="accd0", name="accd0"),
                cst.tile([C, NQ], F32, tag="accd1", name="accd1"),
            ]
            ACCP = cst.tile([C, NQ], F32, tag="accp")
            ACC1 = cst.tile([C, NQ], F32, tag="acc1")
            EL = [None] * MB
            n_dve = 0
            n_pool = 0
            n_pool2 = 0

            def av(i):
                g, u = i // 4, i % 4
                for h in range(2):
                    sl = slice(h * 512, (h + 1) * 512)
                    nc.tensor.matmul(
                        PO[:, sl], V[g][:, u, :], EL[i][:, sl],
                        start=(i == 0), stop=(i == MB - 1),
                    )

            def make_v(g):
                pv = psm.tile([C, 4, 128], F32, tag="ps", name=f"pv{g}", bufs=2)
                for w in range(4):
                    m0 = (g % 2) * 512 + w * 128
                    nc.tensor.matmul(
                        pv[:, w, :],
                        XH[g // 2][:, m0 : m0 + 128],
                        WVF,
                        start=True,
                        stop=True,
                    )
                vt = xp.tile([C, 4, 128], F16, tag=f"v{g}", name=f"v{g}")
                nc.vector.tensor_copy(vt, pv)
                V[g] = vt

            make_v(0)
            for i in range(MB):
                g, u = i // 4, i % 4
                if u == 2 and g + 1 < 2 * NX:
                    make_v(g + 1)
                kblk = K[g][:, u * 128 : (u + 1) * 128]
                psS = psm.tile([C, NQ], F32, tag="psq", name=f"s{i}")
                for h in range(2):
                    sl = slice(h * 512, (h + 1) * 512)
                    nc.tensor.matmul(psS[:, sl], kblk, QT[:, sl], start=True, stop=True)
                if i > 0:
                    av(i - 1)
                E = ep.tile([C, NQ], F16, tag="e", name=f"e{i}")
                nc.scalar.activation(E, psS, Exp, bias=BT[:, i : i + 1])
                EL[i] = E
                # denominator: gpsimd takes every 3rd block (ending early),
                # the vector engine the rest; ACCD[1] covers only the last
                # blocks so the final merge chain is short
                if i % 3 == 2 and i < 27:
                    if n_pool == 0:
                        nc.gpsimd.tensor_copy(ACCP, E)
                    else:
                        nc.gpsimd.tensor_add(ACCP, ACCP, E)
                    n_pool += 1
                else:
                    a = ACCD[1] if i >= 27 else ACCD[0]
                    if i >= 27:
                        if n_pool2 == 0:
                            nc.vector.tensor_copy(a, E)
                        else:
                            nc.vector.tensor_add(a, a, E)
                        n_pool2 += 1
                    else:
                        if n_dve == 0:
                            nc.vector.tensor_copy(a, E)
                        else:
                            nc.vector.tensor_add(a, a, E)
                        n_dve += 1
                if i == 28:
                    # pool (done at 26) + ACCD0 (done at 25) merge early
                    nc.vector.tensor_add(ACC1, ACCD[0], ACCP)
            av(MB - 1)

            # ---- denominator row, 1/d, normalize, project, residual ----
            ACCF = cst.tile([C, NQ], F32R, tag="accf")
            nc.vector.tensor_add(ACCF, ACC1, ACCD[1])
            PD = [
                psm.tile([1, 512], F32, tag="ps", name="pd0", bufs=2),
                psm.tile([1, 512], F32, tag="ps", name="pd1", bufs=2),
            ]
            PDC = cst.tile([1, NQ], F32R, tag="pdc")
            PB = psm.tile([C, NQ], F32, tag="psq", name="pb")
            RB = cst.tile([C, NQ], F32, tag="rb")
            OUTN = cst.tile([C, NQ], F32R, tag="outn")
            PP = psm.tile([C, NQ], F32, tag="psq", name="pp")
            Y = cst.tile([C, NQ], F32, tag="y")
            for h in range(2):
                sl = slice(h * 512, (h + 1) * 512)
                nc.tensor.matmul(PD[h], ONC, ACCF[:, sl], start=True, stop=True)
                nc.scalar.activation(
                    PDC[:, sl], PD[h], mybir.ActivationFunctionType.Copy
                )
                nc.tensor.matmul(PB[:, sl], ONR, PDC[:, sl], start=True, stop=True)
                nc.vector.reciprocal_approx_fast(RB[:, sl], PB[:, sl])
                nc.vector.tensor_mul(OUTN[:, sl], PO[:, sl], RB[:, sl])
                nc.tensor.matmul(PP[:, sl], WPT, OUTN[:, sl], start=True, stop=True)
                nc.vector.tensor_add(Y[:, sl], PP[:, sl], XSB[:, sl])
                nc.sync.dma_start(y_d[:, sl], Y[:, sl])

    nc.compile()
    return nc


def _get_nc():
    if "nc" not in _CACHE:
        _CACHE["nc"] = _build()
    return _CACHE["nc"]


def kernel(
    x,
    gamma,
    beta,
    wq,
    bq,
    wk,
    bk,
    wv,
    bv,
    wp,
    bp,
    _results_hook=None,
    _run_kwargs=None,
    **_unused,
):
    from concourse.bass_utils import run_bass_kernel_spmd

    f = np.float32
    x = np.ascontiguousarray(np.asarray(x, dtype=f))
    Bx, Cx, D, Hh, W = x.shape
    NN = D * Hh * W
    xr = x.reshape(Bx, Cx, NN)

    gamma = np.asarray(gamma, f).reshape(C)
    beta = np.asarray(beta, f).reshape(C)
    wq = np.asarray(wq, f)
    wk = np.asarray(wk, f)
    wv = np.asarray(wv, f)
    wp = np.asarray(wp, f)
    bq = np.asarray(bq, f).reshape(C)
    bv = np.asarray(bv, f).reshape(C)
    bp = np.asarray(bp, f).reshape(C)

    scale = f(1.0) / np.sqrt(f(C))
    gsz = C // GROUPS

    per_batch = []
    for b in range(Bx):
        xg = xr[b].reshape(GROUPS, gsz * NN)
        mean_g = xg.mean(axis=1)
        var_g = xg.var(axis=1)
        s = (gamma.reshape(GROUPS, gsz) / np.sqrt(var_g + f(EPS))[:, None]).reshape(C)
        t = beta - np.repeat(mean_g, gsz) * s
        # fold the groupnorm affine into the weights: W' = W diag(s); b' = W t + b
        wqf = (wq * s[None, :]) * scale
        wkf = wk * s[None, :]
        wvf = wv * s[None, :]
        bqf = (wq @ t + bq) * scale
        bvf = wv @ t + bv
        fb = wp @ bvf + bp  # v-bias contribution + projection bias
        # score bias term (K^T bq'') folded into the exp bias, from raw x
        wstar = wkf.T @ bqf
        bterm = wstar @ xr[b]  # [N]
        wf_blob = np.concatenate([wqf.T, wkf.T, wvf.T], axis=1).astype(np.float16)
        per_batch.append(
            {
                "xh": np.ascontiguousarray(xr[b]).astype(np.float16),
                "wf": np.ascontiguousarray(wf_blob),
                "cols": np.ascontiguousarray(fb[:, None].astype(f)),
                "bt": np.ascontiguousarray(bterm.reshape(MB, C).T.astype(f)),
            }
        )

    shared = {
        "wpt": np.ascontiguousarray(wp.T),
        "onc": np.ones((C, 1), f),
        "onh": np.ones((C, 1), np.float16),
        "onr": np.ones((1, C), f),
    }
    in_maps = []
    for core in range(8):
        b, sq = core // 4, core % 4
        xs = np.ascontiguousarray(xr[b][:, sq * NQ : (sq + 1) * NQ])
        in_maps.append(
            {
                **per_batch[b],
                "xsh": xs.astype(np.float16),
                "xs": xs,
                **shared,
            }
        )

    nc = _get_nc()
    res = run_bass_kernel_spmd(
        nc, in_maps, core_ids=list(range(8)), **(_run_kwargs or {})
    )
    if _results_hook is not None:
        _results_hook(res)

    out = np.empty((Bx, Cx, NN), f)
    for core in range(8):
        b, sq = core // 4, core % 4
        out[b][:, sq * NQ : (sq + 1) * NQ] = res.results[core]["y"]
    return out.reshape(Bx, Cx, D, Hh, W)


# revision 22
# speedup vs baseline: 1.0605x; 1.0605x over previous
"""BottleneckAttention3D kernel for 8 Trainium2 NeuronCores.

Reference computation (per batch b):
    h = GroupNorm(x)                      # [C, N], C=128, N=4096, 8 groups
    q = wq @ h + bq ; k = wk @ h + bk ; v = wv @ h + bv
    attn = softmax(q.T k / sqrt(C))       # [N, N]
    out = v attn.T ; y = x + wp @ out + bp

Sharding: 8 cores = 2 batches x 4 query blocks of NQ=1024 tokens. Each core
computes K/V for its whole batch and Q for its query block, then runs a
flash-attention-style loop over 32 key blocks of 128 tokens; the N^2 score
matrix lives only in PSUM/SBUF.

Host preprocessing (cheap, 0.2% of FLOPs): groupnorm statistics and the
affine fold into the QKV weights (W' = W diag(s), b' = W t + b), plus
weight transposes and fp16 casts of x.

Device-side structure per core:
  * K = Wk' x, V = (Wv' x)^T, q^T = Wq'' x_s + bq'' as fp16 tiles. The K
    bias is dropped entirely: softmax is invariant to per-query shifts.
    The V bias reduces to an additive constant (rows of attn sum to 1),
    folded into the projection bias on host.
  * Main loop (software-pipelined): scoresT block = K-block^T Q (fp16
    matmuls, f32 PSUM) -> exp on ACT (no max subtraction; scores are O(6))
    -> fp16 E tile -> attention*V accumulated in PSUM. The softmax
    denominator sum(E) accumulates on the Vector and GpSimd engines.
  * 1/d = exp(-ln d) on ACT (same table set as Exp), broadcast across
    partitions with a K=1 ones matmul, projection + residual, one DMA out.
"""

import sys

sys.path.insert(0, "/opt/trn_rl_repo")

import numpy as np

B = 2
C = 128
N = 4096  # 16*16*16 tokens
NQ = N // 4  # query block per core (1024)
GROUPS = 8
EPS = 1e-5
XCH = 1024
NX = N // XCH  # 4
MB = N // 128  # 32 key blocks
# denominator accumulation: blocks with (i % 8) < DVE_MOD go to the vector
# engine, the rest to gpsimd
DVE_MOD = 5

_CACHE = {}


def _build():
    import concourse.bacc as bacc
    import concourse.mybir as mybir
    import concourse.tile as tile

    F32 = mybir.dt.float32
    F32R = mybir.dt.float32r
    F16 = mybir.dt.float16
    Exp = mybir.ActivationFunctionType.Exp
    Ln = mybir.ActivationFunctionType.Ln

    nc = bacc.Bacc("TRN2", target_bir_lowering=False, debug=False)

    # ---- DRAM I/O ----
    xh_d = nc.dram_tensor("xh", [C, N], F16, kind="ExternalInput")
    xsh_d = nc.dram_tensor("xsh", [C, NQ], F16, kind="ExternalInput")
    xs_d = nc.dram_tensor("xs", [C, NQ], F32, kind="ExternalInput")
    wf_d = nc.dram_tensor("wf", [C, 3 * C], F16, kind="ExternalInput")  # wq|wk|wv
    wpt_d = nc.dram_tensor("wpt", [C, C], F32R, kind="ExternalInput")
    cols_d = nc.dram_tensor("cols", [C, 1], F32, kind="ExternalInput")  # fb
    bt_d = nc.dram_tensor("bt", [C, MB], F32, kind="ExternalInput")  # k.bq'' per block
    onc_d = nc.dram_tensor("onc", [C, 1], F32R, kind="ExternalInput")
    onh_d = nc.dram_tensor("onh", [C, 1], F16, kind="ExternalInput")
    onr_d = nc.dram_tensor("onr", [1, C], F32R, kind="ExternalInput")
    y_d = nc.dram_tensor("y", [C, NQ], F32, kind="ExternalOutput")

    with tile.TileContext(nc) as tc:
        with (
            tc.tile_pool(name="cst", bufs=1) as cst,
            tc.tile_pool(name="xp", bufs=1) as xp,
            tc.tile_pool(name="ep", bufs=10) as ep,
            tc.tile_pool(name="psm", bufs=2, space="PSUM") as psm,
            tc.tile_pool(name="pso", bufs=1, space="PSUM") as pso,
        ):
            # dummy ACT op: load the ln+exp table set at t=0
            DUM = cst.tile([1, 1], F32, tag="dum")
            nc.vector.memset(DUM, 1.0)
            DUM2 = cst.tile([1, 1], F32, tag="dum2")
            nc.scalar.activation(DUM2, DUM, Exp)

            # ---- input loads ----
            XH = []
            for j in range(NX):
                xt = xp.tile([C, XCH], F16, tag=f"x{j}", name=f"x{j}")
                nc.sync.dma_start(xt, xh_d[:, j * XCH : (j + 1) * XCH])
                XH.append(xt)
            XSH = cst.tile([C, NQ], F16, tag="xsh")
            nc.sync.dma_start(XSH, xsh_d[:, :])
            XS = cst.tile([C, NQ], F32, tag="xs")
            nc.sync.dma_start(XS, xs_d[:, :])
            WF = cst.tile([C, 3 * C], F16, tag="wf")
            nc.gpsimd.dma_start(WF, wf_d[:, :])
            WPT = cst.tile([C, C], F32R, tag="wpt")
            nc.gpsimd.dma_start(WPT, wpt_d[:, :])
            COLS = cst.tile([C, 1], F32, tag="cols")
            nc.gpsimd.dma_start(COLS, cols_d[:, :])
            BT = cst.tile([C, MB], F32, tag="bt")
            nc.gpsimd.dma_start(BT, bt_d[:, :])
            ONC = cst.tile([C, 1], F32R, tag="onc")
            nc.gpsimd.dma_start(ONC, onc_d[:, :])
            ONH = cst.tile([C, 1], F16, tag="onh")
            nc.gpsimd.dma_start(ONH, onh_d[:, :])
            ONR = cst.tile([1, C], F32R, tag="onr")
            nc.gpsimd.dma_start(ONR, onr_d[:, :])
            WQF = WF[:, 0 * C : 1 * C]
            WKF = WF[:, 1 * C : 2 * C]
            WVF = WF[:, 2 * C : 3 * C]
            FB = COLS[:, 0:1]

            # ---- Q then K (fp16; k-bias dropped: softmax shift-invariant) ----
            PQ = psm.tile([C, NQ], F32, tag="psq", name="pq")
            for h in range(2):
                sl = slice(h * 512, (h + 1) * 512)
                nc.tensor.matmul(PQ[:, sl], WQF, XSH[:, sl], start=True, stop=True)
            QT = cst.tile([C, NQ], F16, tag="qt")
            nc.vector.tensor_copy(QT, PQ)
            K = []
            for j2 in range(2 * NX):
                pk = psm.tile([C, 512], F32, tag="ps", name=f"pk{j2}")
                nc.tensor.matmul(
                    pk, WKF, XH[j2 // 2][:, (j2 % 2) * 512 : (j2 % 2 + 1) * 512],
                    start=True, stop=True,
                )
                kt = xp.tile([C, 512], F16, tag=f"k{j2}", name=f"k{j2}")
                nc.vector.tensor_copy(kt, pk)
                K.append(kt)
            V = [None] * (2 * NX)

            # ---- main attention loop ----
            PO = pso.tile([C, NQ], F32, tag="po")
            ACCD = [
                cst.tile([C, NQ], F32, tag="accd0", name="accd0"),
                cst.tile([C, NQ], F32, tag="accd1", name="accd1"),
            ]
            EL = [None] * MB

            def av(i):
                g, u = i // 4, i % 4
                for h in range(2):
                    sl = slice(h * 512, (h + 1) * 512)
                    nc.tensor.matmul(
                        PO[:, sl], V[g][:, u, :], EL[i][:, sl],
                        start=(i == 0), stop=(i == MB - 1),
                    )

            def make_v(g):
                pv = psm.tile([C, 4, 128], F32, tag="ps", name=f"pv{g}", bufs=2)
                for w in range(4):
                    m0 = (g % 2) * 512 + w * 128
                    nc.tensor.matmul(
                        pv[:, w, :],
                        XH[g // 2][:, m0 : m0 + 128],
                        WVF,
                        start=True,
                        stop=True,
                    )
                vt = xp.tile([C, 4, 128], F16, tag=f"v{g}", name=f"v{g}")
                nc.vector.tensor_copy(vt, pv)
                V[g] = vt

            make_v(0)
            for i in range(MB):
                g, u = i // 4, i % 4
                if u == 2 and g + 1 < 2 * NX:
                    make_v(g + 1)
                kblk = K[g][:, u * 128 : (u + 1) * 128]
                psS = psm.tile([C, NQ], F32, tag="psq", name=f"s{i}")
                for h in range(2):
                    sl = slice(h * 512, (h + 1) * 512)
                    nc.tensor.matmul(psS[:, sl], kblk, QT[:, sl], start=True, stop=True)
                if i > 0:
                    av(i - 1)
                E = ep.tile([C, NQ], F16, tag="e", name=f"e{i}")
                nc.scalar.activation(E, psS, Exp, bias=BT[:, i : i + 1])
                EL[i] = E
                # denominator on the vector engine only (gpsimd shares the
                # DVE SBUF port, so pool adds just steal DVE bandwidth);
                # ACCD[1] covers the last 3 blocks to keep the merge short
                a = ACCD[1] if i >= MB - 3 else ACCD[0]
                if i == 0 or i == MB - 3:
                    nc.vector.tensor_copy(a, E)
                else:
                    nc.vector.tensor_add(a, a, E)
            av(MB - 1)

            # ---- denominator row, 1/d, normalize, project, residual ----
            ACCF = cst.tile([C, NQ], F32R, tag="accf")
            nc.vector.tensor_add(ACCF, ACCD[0], ACCD[1])
            XSB = cst.tile([C, NQ], F32, tag="xsb")
            nc.vector.tensor_scalar_add(XSB, XS, FB)
            PD = [
                psm.tile([1, 512], F32, tag="ps", name="pd0", bufs=2),
                psm.tile([1, 512], F32, tag="ps", name="pd1", bufs=2),
            ]
            PDC = cst.tile([1, NQ], F32R, tag="pdc")
            PB = psm.tile([C, NQ], F32, tag="psq", name="pb")
            RB = cst.tile([C, NQ], F32, tag="rb")
            OUTN = cst.tile([C, NQ], F32R, tag="outn")
            PP = psm.tile([C, NQ], F32, tag="psq", name="pp")
            Y = cst.tile([C, NQ], F32, tag="y")
            for h in range(2):
                sl = slice(h * 512, (h + 1) * 512)
                nc.tensor.matmul(PD[h], ONC, ACCF[:, sl], start=True, stop=True)
                nc.scalar.activation(
                    PDC[:, sl], PD[h], mybir.ActivationFunctionType.Copy
                )
                nc.tensor.matmul(PB[:, sl], ONR, PDC[:, sl], start=True, stop=True)
                nc.vector.reciprocal_approx_fast(RB[:, sl], PB[:, sl])
                nc.vector.tensor_mul(OUTN[:, sl], PO[:, sl], RB[:, sl])
                nc.tensor.matmul(PP[:, sl], WPT, OUTN[:, sl], start=True, stop=True)
                nc.vector.tensor_add(Y[:, sl], PP[:, sl], XSB[:, sl])
                nc.sync.dma_start(y_d[:, sl], Y[:, sl])

    nc.compile()
    return nc


def _get_nc():
    if "nc" not in _CACHE:
        _CACHE["nc"] = _build()
    return _CACHE["nc"]


def kernel(
    x,
    gamma,
    beta,
    wq,
    bq,
    wk,
    bk,
    wv,
    bv,
    wp,
    bp,
    _results_hook=None,
    _run_kwargs=None,
    **_unused,
):
    from concourse.bass_utils import run_bass_kernel_spmd

    f = np.float32
    x = np.ascontiguousarray(np.asarray(x, dtype=f))
    Bx, Cx, D, Hh, W = x.shape
    NN = D * Hh * W
    xr = x.reshape(Bx, Cx, NN)

    gamma = np.asarray(gamma, f).reshape(C)
    beta = np.asarray(beta, f).reshape(C)
    wq = np.asarray(wq, f)
    wk = np.asarray(wk, f)
    wv = np.asarray(wv, f)
    wp = np.asarray(wp, f)
    bq = np.asarray(bq, f).reshape(C)
    bv = np.asarray(bv, f).reshape(C)
    bp = np.asarray(bp, f).reshape(C)

    scale = f(1.0) / np.sqrt(f(C))
    gsz = C // GROUPS

    per_batch = []
    for b in range(Bx):
        xg = xr[b].reshape(GROUPS, gsz * NN)
        mean_g = xg.mean(axis=1)
        var_g = xg.var(axis=1)
        s = (gamma.reshape(GROUPS, gsz) / np.sqrt(var_g + f(EPS))[:, None]).reshape(C)
        t = beta - np.repeat(mean_g, gsz) * s
        # fold the groupnorm affine into the weights: W' = W diag(s); b' = W t + b
        wqf = (wq * s[None, :]) * scale
        wkf = wk * s[None, :]
        wvf = wv * s[None, :]
        bqf = (wq @ t + bq) * scale
        bvf = wv @ t + bv
        fb = wp @ bvf + bp  # v-bias contribution + projection bias
        # score bias term (K^T bq'') folded into the exp bias, from raw x
        wstar = wkf.T @ bqf
        bterm = wstar @ xr[b]  # [N]
        wf_blob = np.concatenate([wqf.T, wkf.T, wvf.T], axis=1).astype(np.float16)
        per_batch.append(
            {
                "xh": np.ascontiguousarray(xr[b]).astype(np.float16),
                "wf": np.ascontiguousarray(wf_blob),
                "cols": np.ascontiguousarray(fb[:, None].astype(f)),
                "bt": np.ascontiguousarray(bterm.reshape(MB, C).T.astype(f)),
            }
        )

    shared = {
        "wpt": np.ascontiguousarray(wp.T),
        "onc": np.ones((C, 1), f),
        "onh": np.ones((C, 1), np.float16),
        "onr": np.ones((1, C), f),
    }
    in_maps = []
    for core in range(8):
        b, sq = core // 4, core % 4
        xs = np.ascontiguousarray(xr[b][:, sq * NQ : (sq + 1) * NQ])
        in_maps.append(
            {
                **per_batch[b],
                "xsh": xs.astype(np.float16),
                "xs": xs,
                **shared,
            }
        )

    nc = _get_nc()
    res = run_bass_kernel_spmd(
        nc, in_maps, core_ids=list(range(8)), **(_run_kwargs or {})
    )
    if _results_hook is not None:
        _results_hook(res)

    out = np.empty((Bx, Cx, NN), f)
    for core in range(8):
        b, sq = core // 4, core % 4
        out[b][:, sq * NQ : (sq + 1) * NQ] = res.results[core]["y"]
    return out.reshape(Bx, Cx, D, Hh, W)


# revision 23
# speedup vs baseline: 1.1864x; 1.1187x over previous
"""BottleneckAttention3D kernel for 8 Trainium2 NeuronCores.

Reference computation (per batch b):
    h = GroupNorm(x)                      # [C, N], C=128, N=4096, 8 groups
    q = wq @ h + bq ; k = wk @ h + bk ; v = wv @ h + bv
    attn = softmax(q.T k / sqrt(C))       # [N, N]
    out = v attn.T ; y = x + wp @ out + bp

Sharding: 8 cores = 2 batches x 4 query blocks of NQ=1024 tokens. Each core
computes K/V for its whole batch and Q for its query block, then runs a
flash-attention-style loop over 32 key blocks of 128 tokens; the N^2 score
matrix lives only in PSUM/SBUF.

Host preprocessing (cheap, 0.2% of FLOPs): groupnorm statistics and the
affine fold into the QKV weights (W' = W diag(s), b' = W t + b), plus
weight transposes and fp16 casts of x.

Device-side structure per core:
  * K = Wk' x, V = (Wv' x)^T, q^T = Wq'' x_s + bq'' as fp16 tiles. The K
    bias is dropped entirely: softmax is invariant to per-query shifts.
    The V bias reduces to an additive constant (rows of attn sum to 1),
    folded into the projection bias on host.
  * Main loop (software-pipelined): scoresT block = K-block^T Q (fp16
    matmuls, f32 PSUM) -> exp on ACT (no max subtraction; scores are O(6))
    -> fp16 E tile -> attention*V accumulated in PSUM. The softmax
    denominator sum(E) accumulates on the Vector and GpSimd engines.
  * 1/d = exp(-ln d) on ACT (same table set as Exp), broadcast across
    partitions with a K=1 ones matmul, projection + residual, one DMA out.
"""

import sys

sys.path.insert(0, "/opt/trn_rl_repo")

import numpy as np

B = 2
C = 128
N = 4096  # 16*16*16 tokens
NQ = N // 4  # query block per core (1024)
GROUPS = 8
EPS = 1e-5
XCH = 1024
NX = N // XCH  # 4
MB = N // 128  # 32 key blocks
# denominator accumulation: blocks with (i % 8) < DVE_MOD go to the vector
# engine, the rest to gpsimd
DVE_MOD = 5

_CACHE = {}


def _build():
    import concourse.bacc as bacc
    import concourse.mybir as mybir
    import concourse.tile as tile

    F32 = mybir.dt.float32
    F32R = mybir.dt.float32r
    F16 = mybir.dt.float16
    Exp = mybir.ActivationFunctionType.Exp
    Copy = mybir.ActivationFunctionType.Copy

    nc = bacc.Bacc("TRN2", target_bir_lowering=False, debug=False)

    # ---- DRAM I/O ----
    xh_d = nc.dram_tensor("xh", [C, N], F16, kind="ExternalInput")
    xsh_d = nc.dram_tensor("xsh", [C, NQ], F16, kind="ExternalInput")
    xs_d = nc.dram_tensor("xs", [C, NQ], F32, kind="ExternalInput")
    wf_d = nc.dram_tensor("wf", [C, 3 * C], F16, kind="ExternalInput")  # wq|wk|wv
    wpt_d = nc.dram_tensor("wpt", [C, C], F32R, kind="ExternalInput")
    cols_d = nc.dram_tensor("cols", [C, 1], F32, kind="ExternalInput")  # fb
    bt_d = nc.dram_tensor("bt", [C, MB], F32, kind="ExternalInput")  # k.bq'' per block
    onc_d = nc.dram_tensor("onc", [C, 1], F32R, kind="ExternalInput")
    onh_d = nc.dram_tensor("onh", [C, 1], F16, kind="ExternalInput")
    onr_d = nc.dram_tensor("onr", [1, C], F32R, kind="ExternalInput")
    y_d = nc.dram_tensor("y", [C, NQ], F32, kind="ExternalOutput")

    with tile.TileContext(nc) as tc:
        with (
            tc.tile_pool(name="cst", bufs=1) as cst,
            tc.tile_pool(name="xp", bufs=1) as xp,
            tc.tile_pool(name="ep", bufs=10) as ep,
            tc.tile_pool(name="psm", bufs=2, space="PSUM") as psm,
            tc.tile_pool(name="pso", bufs=1, space="PSUM") as pso,
        ):
            # dummy ACT op: load the ln+exp table set at t=0
            DUM = cst.tile([1, 1], F32, tag="dum")
            nc.vector.memset(DUM, 1.0)
            DUM2 = cst.tile([1, 1], F32, tag="dum2")
            nc.scalar.activation(DUM2, DUM, Exp)

            # ---- input loads ----
            XH = []
            for j in range(NX):
                xt = xp.tile([C, XCH], F16, tag=f"x{j}", name=f"x{j}")
                nc.sync.dma_start(xt, xh_d[:, j * XCH : (j + 1) * XCH])
                XH.append(xt)
            XSH = cst.tile([C, NQ], F16, tag="xsh")
            nc.sync.dma_start(XSH, xsh_d[:, :])
            XS = cst.tile([C, NQ], F32, tag="xs")
            nc.sync.dma_start(XS, xs_d[:, :])
            WF = cst.tile([C, 3 * C], F16, tag="wf")
            nc.gpsimd.dma_start(WF, wf_d[:, :])
            WPT = cst.tile([C, C], F32R, tag="wpt")
            nc.gpsimd.dma_start(WPT, wpt_d[:, :])
            COLS = cst.tile([C, 1], F32, tag="cols")
            nc.gpsimd.dma_start(COLS, cols_d[:, :])
            BT = cst.tile([C, MB], F32, tag="bt")
            nc.gpsimd.dma_start(BT, bt_d[:, :])
            ONC = cst.tile([C, 1], F32R, tag="onc")
            nc.gpsimd.dma_start(ONC, onc_d[:, :])
            ONH = cst.tile([C, 1], F16, tag="onh")
            nc.gpsimd.dma_start(ONH, onh_d[:, :])
            ONR = cst.tile([1, C], F32R, tag="onr")
            nc.gpsimd.dma_start(ONR, onr_d[:, :])
            WQF = WF[:, 0 * C : 1 * C]
            WKF = WF[:, 1 * C : 2 * C]
            WVF = WF[:, 2 * C : 3 * C]
            FB = COLS[:, 0:1]

            # ---- Q then K (fp16; k-bias dropped: softmax shift-invariant) ----
            PQ = psm.tile([C, NQ], F32, tag="psq", name="pq")
            for h in range(2):
                sl = slice(h * 512, (h + 1) * 512)
                nc.tensor.matmul(PQ[:, sl], WQF, XSH[:, sl], start=True, stop=True)
            QT = cst.tile([C, NQ], F16, tag="qt")
            nc.scalar.activation(QT, PQ, Copy)
            K = []
            for j2 in range(2 * NX):
                pk = psm.tile([C, 512], F32, tag="ps", name=f"pk{j2}")
                nc.tensor.matmul(
                    pk, WKF, XH[j2 // 2][:, (j2 % 2) * 512 : (j2 % 2 + 1) * 512],
                    start=True, stop=True,
                )
                kt = xp.tile([C, 512], F16, tag=f"k{j2}", name=f"k{j2}")
                nc.scalar.activation(kt, pk, Copy)
                K.append(kt)
            V = [None] * (2 * NX)

            # ---- main attention loop ----
            PO = pso.tile([C, NQ], F32, tag="po")
            ACCF = cst.tile([C, NQ], F32R, tag="accf")
            EL = [None] * MB
            PD = [None, None]

            def av(i):
                g, u = i // 4, i % 4
                for h in range(2):
                    sl = slice(h * 512, (h + 1) * 512)
                    nc.tensor.matmul(
                        PO[:, sl], V[g][:, u, :], EL[i][:, sl],
                        start=(i == 0), stop=(i == MB - 1),
                    )

            def make_v(g):
                pv = psm.tile([C, 4, 128], F32, tag="ps", name=f"pv{g}", bufs=2)
                for w in range(4):
                    m0 = (g % 2) * 512 + w * 128
                    nc.tensor.matmul(
                        pv[:, w, :],
                        XH[g // 2][:, m0 : m0 + 128],
                        WVF,
                        start=True,
                        stop=True,
                    )
                vt = xp.tile([C, 4, 128], F16, tag=f"v{g}", name=f"v{g}")
                nc.vector.tensor_copy(vt, pv)
                V[g] = vt

            make_v(0)
            for i in range(MB):
                g, u = i // 4, i % 4
                if u == 2 and g + 1 < 2 * NX:
                    make_v(g + 1)
                kblk = K[g][:, u * 128 : (u + 1) * 128]
                psS = psm.tile([C, NQ], F32, tag="psq", name=f"s{i}")
                for h in range(2):
                    sl = slice(h * 512, (h + 1) * 512)
                    nc.tensor.matmul(psS[:, sl], kblk, QT[:, sl], start=True, stop=True)
                if i > 0:
                    av(i - 1)
                E = ep.tile([C, NQ], F16, tag="e", name=f"e{i}")
                nc.scalar.activation(E, psS, Exp, bias=BT[:, i : i + 1])
                EL[i] = E
                # denominator: vector engine for blocks 0..27 (gpsimd would
                # steal the shared DVE SBUF port), PE ones-matmuls into PSUM
                # for the last 4 so no merge chain trails the loop
                if i < MB - 4:
                    if i == 0:
                        nc.vector.tensor_copy(ACCF, E)
                    else:
                        nc.vector.tensor_add(ACCF, ACCF, E)
                else:
                    if i == MB - 4:
                        PD[0] = psm.tile([1, 512], F32, tag="ps", name="pd0", bufs=2)
                        PD[1] = psm.tile([1, 512], F32, tag="ps", name="pd1", bufs=2)
                    for h in range(2):
                        sl = slice(h * 512, (h + 1) * 512)
                        nc.tensor.matmul(
                            PD[h], ONH, E[:, sl],
                            start=(i == MB - 4), stop=False,
                        )
            av(MB - 1)

            # ---- denominator row, 1/d, normalize, project, residual ----
            XSB = cst.tile([C, NQ], F32, tag="xsb")
            nc.vector.tensor_scalar_add(XSB, XS, FB)
            PDC = cst.tile([1, NQ], F32R, tag="pdc")
            PB = psm.tile([C, NQ], F32, tag="psq", name="pb")
            RB = cst.tile([C, NQ], F32, tag="rb")
            OUTN = cst.tile([C, NQ], F32R, tag="outn")
            PP = psm.tile([C, NQ], F32, tag="psq", name="pp")
            Y = cst.tile([C, NQ], F32, tag="y")
            for h in range(2):
                sl = slice(h * 512, (h + 1) * 512)
                nc.tensor.matmul(PD[h], ONC, ACCF[:, sl], start=False, stop=True)
                nc.scalar.activation(
                    PDC[:, sl], PD[h], mybir.ActivationFunctionType.Copy
                )
                nc.tensor.matmul(PB[:, sl], ONR, PDC[:, sl], start=True, stop=True)
                nc.vector.reciprocal_approx_fast(RB[:, sl], PB[:, sl])
                nc.vector.tensor_mul(OUTN[:, sl], PO[:, sl], RB[:, sl])
                nc.tensor.matmul(PP[:, sl], WPT, OUTN[:, sl], start=True, stop=True)
                nc.vector.tensor_add(Y[:, sl], PP[:, sl], XSB[:, sl])
                nc.sync.dma_start(y_d[:, sl], Y[:, sl])

    nc.compile()
    return nc


def _get_nc():
    if "nc" not in _CACHE:
        _CACHE["nc"] = _build()
    return _CACHE["nc"]


def kernel(
    x,
    gamma,
    beta,
    wq,
    bq,
    wk,
    bk,
    wv,
    bv,
    wp,
    bp,
    _results_hook=None,
    _run_kwargs=None,
    **_unused,
):
    from concourse.bass_utils import run_bass_kernel_spmd

    f = np.float32
    x = np.ascontiguousarray(np.asarray(x, dtype=f))
    Bx, Cx, D, Hh, W = x.shape
    NN = D * Hh * W
    xr = x.reshape(Bx, Cx, NN)

    gamma = np.asarray(gamma, f).reshape(C)
    beta = np.asarray(beta, f).reshape(C)
    wq = np.asarray(wq, f)
    wk = np.asarray(wk, f)
    wv = np.asarray(wv, f)
    wp = np.asarray(wp, f)
    bq = np.asarray(bq, f).reshape(C)
    bv = np.asarray(bv, f).reshape(C)
    bp = np.asarray(bp, f).reshape(C)

    scale = f(1.0) / np.sqrt(f(C))
    gsz = C // GROUPS

    per_batch = []
    for b in range(Bx):
        xg = xr[b].reshape(GROUPS, gsz * NN)
        mean_g = xg.mean(axis=1)
        var_g = xg.var(axis=1)
        s = (gamma.reshape(GROUPS, gsz) / np.sqrt(var_g + f(EPS))[:, None]).reshape(C)
        t = beta - np.repeat(mean_g, gsz) * s
        # fold the groupnorm affine into the weights: W' = W diag(s); b' = W t + b
        wqf = (wq * s[None, :]) * scale
        wkf = wk * s[None, :]
        wvf = wv * s[None, :]
        bqf = (wq @ t + bq) * scale
        bvf = wv @ t + bv
        fb = wp @ bvf + bp  # v-bias contribution + projection bias
        # score bias term (K^T bq'') folded into the exp bias, from raw x
        wstar = wkf.T @ bqf
        bterm = wstar @ xr[b]  # [N]
        wf_blob = np.concatenate([wqf.T, wkf.T, wvf.T], axis=1).astype(np.float16)
        per_batch.append(
            {
                "xh": np.ascontiguousarray(xr[b]).astype(np.float16),
                "wf": np.ascontiguousarray(wf_blob),
                "cols": np.ascontiguousarray(fb[:, None].astype(f)),
                "bt": np.ascontiguousarray(bterm.reshape(MB, C).T.astype(f)),
            }
        )

    shared = {
        "wpt": np.ascontiguousarray(wp.T),
        "onc": np.ones((C, 1), f),
        "onh": np.ones((C, 1), np.float16),
        "onr": np.ones((1, C), f),
    }
    in_maps = []
    for core in range(8):
        b, sq = core // 4, core % 4
        xs = np.ascontiguousarray(xr[b][:, sq * NQ : (sq + 1) * NQ])
        in_maps.append(
            {
                **per_batch[b],
                "xsh": xs.astype(np.float16),
                "xs": xs,
                **shared,
            }
        )

    nc = _get_nc()
    res = run_bass_kernel_spmd(
        nc, in_maps, core_ids=list(range(8)), **(_run_kwargs or {})
    )
    if _results_hook is not None:
        _results_hook(res)

    out = np.empty((Bx, Cx, NN), f)
    for core in range(8):
        b, sq = core // 4, core % 4
        out[b][:, sq * NQ : (sq + 1) * NQ] = res.results[core]["y"]
    return out.reshape(Bx, Cx, D, Hh, W)
